# revision 3
# baseline (speedup 1.0000x reference)
"""BandSplitModule Trainium2 kernel.

Strategy (band/expert parallel, per spec sharding_hint): the 41 (band,
subband) groups are distributed across the 8 NeuronCores, balanced by
total HBM traffic (input bins + weights + output).  Each core runs its
own small Bass/Tile program over host-staged, densely packed inputs:

  per data tile [128 bins x 1024 (t,k)]:
    - bn_stats/bn_aggr       -> per-bin mean / E[x^2]   (VectorE, 1 pass)
    - fp32 indicator matmul  -> per-group sums of (m1, m2) in PSUM
    - 2x float32r matmuls    -> accumulate W'^T x into the group PSUM
  per (b, group) epilogue: var/rsqrt on-chip, broadcast via K=1 matmul,
    out = r * psum + (bias' - mu*r*g1)  (ScalarE), DMA to staging.

GroupNorm is folded into the conv algebraically: with W' = gamma*W,
  y = r*(W'x) + [bias + sum(beta*W) - mu*r*sum(gamma*W)]
so the normalization needs only per-group (mean, E[x^2]) scalars and the
data is read exactly once.  The host reassembles the full
[2, 128, 41, 512] output (the torch channel-order scramble is a reshape).
"""

import threading

import numpy as np

import concourse.bass as bass
import concourse.mybir as mybir
import concourse.tile as tile

SR = 44100
BANDS = [0, 1000, 4000, 8000, 16000, 20000, 22050]
SUB = [10, 12, 8, 8, 2, 1]
FC = 128
NB = len(SUB)
RANGES = [BANDS[i + 1] - BANDS[i] for i in range(NB)]
WIDTHS = [RANGES[i] // SUB[i] for i in range(NB)]
B, T = 2, 512
EPS = 1e-5
GSTART = np.concatenate([[0], np.cumsum(SUB)]).astype(int)  # global subband index base

F32 = mybir.dt.float32
F32R = mybir.dt.float32r


# ----------------------------------------------------------------------------
# Partition: per-core list of (band, subband) groups, balanced by traffic.
# ----------------------------------------------------------------------------
def _build_layouts():
    """Returns per-core list of dicts: {band, sub, w, n_tiles}."""
    # group inventory by band: 0:(100 x10) 1:(250 x12) 2:(500 x8) 3:(1000 x8)
    # 4:(2000 x2) 5:(2050 x1)
    per_core_bands = [
        [5, 2, 0, 0],
        [4, 2, 1, 0],
        [4, 2, 1, 0],
        [3, 3, 2, 1],
        [3, 3, 2, 1],
        [3, 3, 2, 1, 0],
        [3, 3, 2, 1, 0],
        [2, 1, 1, 1, 1, 1, 1, 0, 0, 0, 0],
    ]
    next_sub = [0] * NB
    layouts = []
    for bands in per_core_bands:
        groups = []
        for i in bands:
            s = next_sub[i]
            next_sub[i] += 1
            w = WIDTHS[i]
            groups.append(dict(band=i, sub=s, w=w, n_tiles=(w + 127) // 128))
        layouts.append(groups)
    assert next_sub == SUB, (next_sub, SUB)
    return layouts


LAYOUTS = _build_layouts()


# ----------------------------------------------------------------------------
# Workaround for this container's walrus build: it rejects instructions that
# carry multiple semaphore waits, and any wait on Drain/Matmult.  Move those
# waits onto inserted EventSemaphore instructions (one wait each).
# ----------------------------------------------------------------------------
def _fixup_waits(nc):
    def keep(ins):
        return 0 if ins.opcode in ("Drain", "Matmult") else 1

    for f in nc.m.functions:
        for bb in f.blocks:
            insts = list(bb.instructions)
            if not any(
                i.sync_info is not None
                and len(i.sync_info.on_wait) > keep(i)
                and i.opcode != "EventSemaphore"
                for i in insts
            ):
                continue
            out = []
            for ins in insts:
                si = ins.sync_info
                k = keep(ins)
                if (
                    si is not None
                    and len(si.on_wait) > k
                    and ins.opcode != "EventSemaphore"
                ):
                    waits = list(si.on_wait)
                    for j, w in enumerate(waits[k:]):
                        ev = mybir.InstEventSemaphore(name=f"{ins.name}-dw{j}")
                        ev.engine = ins.engine
                        ev.sync_info = mybir.SyncInfo(on_wait=[w], on_update=[])
                        nc.register_instruction(ev)
                        out.append(ev)
                    ins.sync_info = mybir.SyncInfo(
                        on_wait=waits[:k], on_update=list(si.on_update)
                    )
                out.append(ins)
            bb.instructions = out


# ----------------------------------------------------------------------------
# Per-core Bass program.
# ----------------------------------------------------------------------------
def _build_core_program(groups):
    import contextlib

    NT = sum(g["n_tiles"] for g in groups)
    G = len(groups)

    nc = bass.Bass("TRN2", target_bir_lowering=False)
    xdat = nc.dram_tensor("xdat", [B, NT, 128, 1024], F32R, kind="ExternalInput")
    wdat = nc.dram_tensor("wdat", [NT, 2, 128, 128], F32R, kind="ExternalInput")
    seldat = nc.dram_tensor("seldat", [NT, 128, G], F32, kind="ExternalInput")
    bias2d = nc.dram_tensor("bias2d", [128, G], F32, kind="ExternalInput")
    g1negd = nc.dram_tensor("g1negd", [128, G], F32, kind="ExternalInput")
    invcd = nc.dram_tensor("invcd", [1, G], F32, kind="ExternalInput")
    ydat = nc.dram_tensor("ydat", [B, G, 128, 512], F32, kind="ExternalOutput")

    with tile.TileContext(nc) as tc, contextlib.ExitStack() as ctx:
        consts = ctx.enter_context(tc.tile_pool(name="consts", bufs=1))
        xpool = ctx.enter_context(tc.tile_pool(name="xpool", bufs=4))
        spool = ctx.enter_context(tc.tile_pool(name="spool", bufs=4))
        yopool = ctx.enter_context(tc.tile_pool(name="yopool", bufs=3))
        gpool = ctx.enter_context(tc.tile_pool(name="gpool", bufs=2))
        mainps = ctx.enter_context(tc.tile_pool(name="mainps", bufs=3, space="PSUM"))
        statps = ctx.enter_context(tc.tile_pool(name="statps", bufs=2, space="PSUM"))
        bcps = ctx.enter_context(tc.tile_pool(name="bcps", bufs=2, space="PSUM"))

        wbuf = consts.tile([128, NT, 2, 128], F32R)
        # DRAM wdat is [NT, 2, 128(bins), 128(f)]; SBUF partition = bins.
        nc.sync.dma_start(out=wbuf, in_=wdat.rearrange("nt k p f -> p nt k f"))
        selbuf = consts.tile([128, NT, G], F32)
        nc.sync.dma_start(out=selbuf, in_=seldat.rearrange("nt p g -> p nt g"))
        bias2 = consts.tile([128, G], F32)
        nc.sync.dma_start(out=bias2, in_=bias2d[:, :])
        g1neg = consts.tile([128, G], F32)
        nc.sync.dma_start(out=g1neg, in_=g1negd[:, :])
        invc = consts.tile([1, G], F32)
        nc.sync.dma_start(out=invc, in_=invcd[:, :])
        ones_t = consts.tile([1, 128], F32)
        nc.vector.memset(ones_t, 1.0)
        eps_t = consts.tile([1, 1], F32)
        nc.vector.memset(eps_t, EPS)

        for b in range(B):
            j = 0
            for gl, grp in enumerate(groups):
                ntg = grp["n_tiles"]
                ps_main = mainps.tile([128, 512], F32, tag="ps_main")
                ps_statg = statps.tile([1, 2], F32, tag="ps_statg")
                for c in range(ntg):
                    xt = xpool.tile([128, 1024], F32R, tag="xt")
                    nc.sync.dma_start(out=xt, in_=xdat[b, j])
                    xf = xt.bitcast(F32)
                    st = spool.tile([128, 12], F32, tag="st")
                    nc.vector.bn_stats(out=st[:, 0:6], in_=xf[:, 0:512])
                    nc.vector.bn_stats(out=st[:, 6:12], in_=xf[:, 512:1024])
                    mv = spool.tile([128, 2], F32, tag="mv")
                    nc.vector.bn_aggr(out=mv, in_=st)
                    # mv -> [mean, E[x^2]] per bin
                    nc.vector.scalar_tensor_tensor(
                        out=mv[:, 1:2],
                        in0=mv[:, 0:1],
                        scalar=mv[:, 0:1],
                        in1=mv[:, 1:2],
                        op0=mybir.AluOpType.mult,
                        op1=mybir.AluOpType.add,
                    )
                    # group sums of (m1, m2): psum[1, 2] += sel_g.T @ mv
                    nc.tensor.matmul(
                        ps_statg[:, :],
                        selbuf[:, j, gl : gl + 1],
                        mv[:, :],
                        start=(c == 0),
                        stop=(c == ntg - 1),
                    )
                    # main: psum[128 f, 512 t] += W'.T @ x  (k = 0, 1)
                    x3 = xt.rearrange("p (t k) -> p t k", k=2)
                    for k in range(2):
                        nc.tensor.matmul(
                            ps_main[:, :],
                            wbuf[:, j, k, :],
                            x3[:, :, k],
                            start=(c == 0 and k == 0),
                            stop=(c == ntg - 1 and k == 1),
                        )
                    j += 1

                # ---- epilogue for (b, gl), all scalars on partition 0 ----
                mue = gpool.tile([1, 2], F32, tag="mue")
                tmp1 = gpool.tile([1, 1], F32, tag="tmp1")
                tmp2 = gpool.tile([1, 1], F32, tag="tmp2")
                rm = gpool.tile([1, 2], F32, tag="rm")
                # mu, e2 = psum_stats / w
                nc.scalar.activation(
                    mue[:, :],
                    ps_statg[:, :],
                    mybir.ActivationFunctionType.Copy,
                    bias=0.0,
                    scale=invc[0:1, gl : gl + 1],
                )
                # -var = mu*mu - e2
                nc.vector.scalar_tensor_tensor(
                    out=tmp1[:, :],
                    in0=mue[:, 0:1],
                    scalar=mue[:, 0:1],
                    in1=mue[:, 1:2],
                    op0=mybir.AluOpType.mult,
                    op1=mybir.AluOpType.subtract,
                )
                # sd = sqrt(var + eps)
                nc.scalar.activation(
                    tmp2[:, :],
                    tmp1[:, :],
                    mybir.ActivationFunctionType.Sqrt,
                    bias=eps_t[0:1, 0:1],
                    scale=-1.0,
                )
                # r = 1/sd ; mur = mu*r
                nc.vector.reciprocal(out=rm[:, 0:1], in_=tmp2[:, :])
                nc.vector.tensor_scalar_mul(
                    out=rm[:, 1:2], in0=mue[:, 0:1], scalar1=rm[:, 0:1]
                )
                # broadcast (r, mur) to 128 partitions via K=1 fp32 matmul
                ps_bc = bcps.tile([128, 2], F32, tag="ps_bc")
                nc.tensor.matmul(
                    ps_bc[:, :], ones_t[:, :], rm[:, :], start=True, stop=True
                )
                rb = gpool.tile([128, 2], F32, tag="rb")
                nc.vector.tensor_copy(rb, ps_bc[:, :])
                # d = g1neg * mur + bias2
                d_t = gpool.tile([128, 1], F32, tag="d_t")
                nc.vector.scalar_tensor_tensor(
                    out=d_t,
                    in0=g1neg[:, gl : gl + 1],
                    scalar=rb[:, 1:2],
                    in1=bias2[:, gl : gl + 1],
                    op0=mybir.AluOpType.mult,
                    op1=mybir.AluOpType.add,
                )
                # y = r*psum + d
                yo = yopool.tile([128, 512], F32, tag="yo")
                nc.scalar.activation(
                    yo,
                    ps_main[:, :],
                    mybir.ActivationFunctionType.Identity,
                    bias=d_t[:, 0:1],
                    scale=rb[:, 0:1],
                )
                nc.sync.dma_start(out=ydat[b, gl], in_=yo)

    _fixup_waits(nc)
    return nc


# ----------------------------------------------------------------------------
# Host staging.
# ----------------------------------------------------------------------------
def _stage_core_inputs(core, cspec, gn_gamma, gn_beta, conv_w, conv_b):
    groups = LAYOUTS[core]
    NT = sum(g["n_tiles"] for g in groups)
    G = len(groups)

    xdat = np.zeros((B, NT, 128, 1024), np.float32)
    wdat = np.zeros((NT, 2, 128, 128), np.float32)
    sel = np.zeros((NT, 128, G), np.float32)
    bias2 = np.zeros((128, G), np.float32)
    g1neg = np.zeros((128, G), np.float32)
    invc = np.zeros((1, G), np.float32)

    j = 0
    for gl, grp in enumerate(groups):
        i, s, w = grp["band"], grp["sub"], grp["w"]
        base = BANDS[i] + s * w
        gam = np.asarray(gn_gamma[i][s * w : (s + 1) * w], np.float32)  # [w]
        bet = np.asarray(gn_beta[i][s * w : (s + 1) * w], np.float32)
        Wg = np.asarray(conv_w[i][s * FC : (s + 1) * FC], np.float32)  # [FC, w, 2]
        bg = np.asarray(conv_b[i][s * FC : (s + 1) * FC], np.float32)  # [FC]
        Wp = Wg * gam[None, :, None]  # gamma-folded [FC, w, 2]
        bias2[:, gl] = bg + (Wg * bet[None, :, None]).sum(axis=(1, 2))
        g1neg[:, gl] = -Wp.sum(axis=(1, 2))
        invc[0, gl] = 1.0 / w
        for c in range(grp["n_tiles"]):
            r0, r1 = c * 128, min((c + 1) * 128, w)
            rows = r1 - r0
            for b in range(B):
                xdat[b, j, :rows, :] = cspec[b, base + r0 : base + r1].reshape(
                    rows, 1024
                )
            # wdat[j, k, r, f] = Wp[f, r0+r, k]
            wdat[j, :, :rows, :] = Wp[:, r0:r1, :].transpose(2, 1, 0)
            sel[j, :rows, gl] = 1.0
            j += 1
    return dict(
        xdat=xdat, wdat=wdat, seldat=sel, bias2d=bias2, g1negd=g1neg, invcd=invc
    )


def _assemble_output(core_outs):
    """core_outs: list of ydat arrays [B, G, 128, 512] -> full [B, FC, 41, T]."""
    out = np.zeros((B, FC, int(GSTART[-1]), T), np.float32)
    band_blocks = [[None] * SUB[i] for i in range(NB)]  # band -> sub -> [B,128,512]
    for core, groups in enumerate(LAYOUTS):
        for gl, grp in enumerate(groups):
            band_blocks[grp["band"]][grp["sub"]] = core_outs[core][:, gl]
    for i in range(NB):
        s_i = SUB[i]
        yb = np.stack(band_blocks[i], axis=1)  # [B, s_i, 128, 512]
        # torch order: flat (s*FC) viewed as (FC, s)
        out[:, :, GSTART[i] : GSTART[i] + s_i, :] = yb.reshape(B, s_i * FC, T).reshape(
            B, FC, s_i, T
        )
    return out


# ----------------------------------------------------------------------------
# Runner: compile the 8 per-core programs (cached), run one per device.
# ----------------------------------------------------------------------------
_CACHE = {}
_CACHE_LOCK = threading.Lock()


def _make_fn(nc):
    import jax
    import jax.core as jcore

    from concourse import bass2jax

    bass2jax.install_neuronx_cc_hook()

    in_names, out_names, out_avals, zero_outs = [], [], [], []
    partition_name = nc.partition_id_tensor.name if nc.partition_id_tensor else None
    for alloc in nc.m.functions[0].allocations:
        if not isinstance(alloc, mybir.MemoryLocationSet):
            continue
        name = alloc.memorylocations[0].name
        if alloc.kind == "ExternalInput":
            if name != partition_name:
                in_names.append(name)
        elif alloc.kind == "ExternalOutput":
            out_names.append(name)
            shape = tuple(alloc.tensor_shape)
            dtype = mybir.dt.np(alloc.dtype)
            out_avals.append(jcore.ShapedArray(shape, dtype))
            zero_outs.append(np.zeros(shape, dtype))
    n_params = len(in_names)
    all_in = list(in_names) + list(out_names)
    if partition_name is not None:
        all_in.append(partition_name)
    donate = tuple(range(n_params, n_params + len(out_names)))

    def _body(*args):
        operands = list(args)
        if partition_name is not None:
            operands.append(bass2jax.partition_id_tensor())
        outs = bass2jax._bass_exec_p.bind(
            *operands,
            out_avals=tuple(out_avals),
            in_names=tuple(all_in),
            out_names=tuple(out_names),
            lowering_input_output_aliases=(),
            sim_require_finite=False,
            sim_require_nnan=False,
            nc=nc,
        )
        return tuple(outs)

    fn = jax.jit(_body, donate_argnums=donate, keep_unused=True)
    return fn, in_names, out_names, zero_outs


def _get_programs():
    with _CACHE_LOCK:
        if "fns" not in _CACHE:
            ncs = [_build_core_program(LAYOUTS[c]) for c in range(8)]
            _CACHE["fns"] = [_make_fn(nc) for nc in ncs]
    return _CACHE["fns"]


def run_cores(core_in_maps):
    """Dispatch the 8 per-core programs on the 8 devices; returns ydat list."""
    import jax

    fns = _get_programs()
    devs = jax.devices()[:8]
    futs = []
    for i in range(8):
        fn, in_names, out_names, zero_outs = fns[i]
        args = [jax.device_put(core_in_maps[i][n], devs[i]) for n in in_names]
        args += [jax.device_put(z, devs[i]) for z in zero_outs]
        futs.append(fn(*args))
    for f in futs:
        jax.block_until_ready(f)
    return [np.asarray(futs[i][0]) for i in range(8)]


def kernel(cspec, gn_gamma, gn_beta, conv_w, conv_b):
    cspec = np.asarray(cspec, np.float32)
    in_maps = [
        _stage_core_inputs(c, cspec, gn_gamma, gn_beta, conv_w, conv_b)
        for c in range(8)
    ]
    core_outs = run_cores(in_maps)
    return _assemble_output(core_outs)


# revision 4
# speedup vs baseline: 1.0011x; 1.0011x over previous
"""BandSplitModule Trainium2 kernel.

Strategy (band/expert parallel, per spec sharding_hint): the 41 (band,
subband) groups are distributed across the 8 NeuronCores, balanced by
total HBM traffic (input bins + weights + output).  Each core runs its
own small Bass/Tile program over host-staged, densely packed inputs:

  per data tile [128 bins x 1024 (t,k)]:
    - bn_stats/bn_aggr       -> per-bin mean / E[x^2]   (VectorE, 1 pass)
    - fp32 indicator matmul  -> per-group sums of (m1, m2) in PSUM
    - 2x float32r matmuls    -> accumulate W'^T x into the group PSUM
  per (b, group) epilogue: var/rsqrt on-chip, broadcast via K=1 matmul,
    out = r * psum + (bias' - mu*r*g1)  (ScalarE), DMA to staging.

GroupNorm is folded into the conv algebraically: with W' = gamma*W,
  y = r*(W'x) + [bias + sum(beta*W) - mu*r*sum(gamma*W)]
so the normalization needs only per-group (mean, E[x^2]) scalars and the
data is read exactly once.  The host reassembles the full
[2, 128, 41, 512] output (the torch channel-order scramble is a reshape).
"""

import threading

import numpy as np

import concourse.bass as bass
import concourse.mybir as mybir
import concourse.tile as tile

SR = 44100
BANDS = [0, 1000, 4000, 8000, 16000, 20000, 22050]
SUB = [10, 12, 8, 8, 2, 1]
FC = 128
NB = len(SUB)
RANGES = [BANDS[i + 1] - BANDS[i] for i in range(NB)]
WIDTHS = [RANGES[i] // SUB[i] for i in range(NB)]
B, T = 2, 512
EPS = 1e-5
GSTART = np.concatenate([[0], np.cumsum(SUB)]).astype(int)  # global subband index base

F32 = mybir.dt.float32
F32R = mybir.dt.float32r


# ----------------------------------------------------------------------------
# Partition: per-core list of (band, subband) groups, balanced by traffic.
# ----------------------------------------------------------------------------
def _build_layouts():
    """Returns per-core list of dicts: {band, sub, w, n_tiles}."""
    # group inventory by band: 0:(100 x10) 1:(250 x12) 2:(500 x8) 3:(1000 x8)
    # 4:(2000 x2) 5:(2050 x1)
    per_core_bands = [
        [5, 2, 0, 0],
        [4, 2, 1, 0],
        [4, 2, 1, 0],
        [3, 3, 2, 1],
        [3, 3, 2, 1],
        [3, 3, 2, 1, 0],
        [3, 3, 2, 1, 0],
        [2, 1, 1, 1, 1, 1, 1, 0, 0, 0, 0],
    ]
    next_sub = [0] * NB
    layouts = []
    for bands in per_core_bands:
        groups = []
        for i in bands:
            s = next_sub[i]
            next_sub[i] += 1
            w = WIDTHS[i]
            groups.append(dict(band=i, sub=s, w=w, n_tiles=(w + 127) // 128))
        layouts.append(groups)
    assert next_sub == SUB, (next_sub, SUB)
    return layouts


LAYOUTS = _build_layouts()


# ----------------------------------------------------------------------------
# Workaround for this container's walrus build: it rejects instructions that
# carry multiple semaphore waits, and any wait on Drain/Matmult.  Move those
# waits onto inserted EventSemaphore instructions (one wait each).
# ----------------------------------------------------------------------------
def _fixup_waits(nc):
    def keep(ins):
        return 0 if ins.opcode in ("Drain", "Matmult") else 1

    for f in nc.m.functions:
        for bb in f.blocks:
            insts = list(bb.instructions)
            if not any(
                i.sync_info is not None
                and len(i.sync_info.on_wait) > keep(i)
                and i.opcode != "EventSemaphore"
                for i in insts
            ):
                continue
            out = []
            for ins in insts:
                si = ins.sync_info
                k = keep(ins)
                if (
                    si is not None
                    and len(si.on_wait) > k
                    and ins.opcode != "EventSemaphore"
                ):
                    waits = list(si.on_wait)
                    for j, w in enumerate(waits[k:]):
                        ev = mybir.InstEventSemaphore(name=f"{ins.name}-dw{j}")
                        ev.engine = ins.engine
                        ev.sync_info = mybir.SyncInfo(on_wait=[w], on_update=[])
                        nc.register_instruction(ev)
                        out.append(ev)
                    ins.sync_info = mybir.SyncInfo(
                        on_wait=waits[:k], on_update=list(si.on_update)
                    )
                out.append(ins)
            bb.instructions = out


# ----------------------------------------------------------------------------
# Per-core Bass program.
# ----------------------------------------------------------------------------
def _build_core_program(groups):
    import contextlib

    NT = sum(g["n_tiles"] for g in groups)
    G = len(groups)

    nc = bass.Bass("TRN2", target_bir_lowering=False)
    xdat = nc.dram_tensor("xdat", [B, NT, 128, 1024], F32R, kind="ExternalInput")
    wdat = nc.dram_tensor("wdat", [NT, 2, 128, 128], F32R, kind="ExternalInput")
    seldat = nc.dram_tensor("seldat", [NT, 128, G], F32, kind="ExternalInput")
    bias2d = nc.dram_tensor("bias2d", [128, G], F32, kind="ExternalInput")
    g1negd = nc.dram_tensor("g1negd", [128, G], F32, kind="ExternalInput")
    invcd = nc.dram_tensor("invcd", [1, G], F32, kind="ExternalInput")
    ydat = nc.dram_tensor("ydat", [B, G, 128, 512], F32, kind="ExternalOutput")

    with tile.TileContext(nc) as tc, contextlib.ExitStack() as ctx:
        consts = ctx.enter_context(tc.tile_pool(name="consts", bufs=1))
        xpool = ctx.enter_context(tc.tile_pool(name="xpool", bufs=4))
        spool = ctx.enter_context(tc.tile_pool(name="spool", bufs=4))
        yopool = ctx.enter_context(tc.tile_pool(name="yopool", bufs=3))
        gpool = ctx.enter_context(tc.tile_pool(name="gpool", bufs=2))
        mainps = ctx.enter_context(tc.tile_pool(name="mainps", bufs=3, space="PSUM"))
        statps = ctx.enter_context(tc.tile_pool(name="statps", bufs=2, space="PSUM"))
        bcps = ctx.enter_context(tc.tile_pool(name="bcps", bufs=2, space="PSUM"))

        wbuf = consts.tile([128, NT, 2, 128], F32R)
        # DRAM wdat is [NT, 2, 128(bins), 128(f)]; SBUF partition = bins.
        nc.sync.dma_start(out=wbuf, in_=wdat.rearrange("nt k p f -> p nt k f"))
        selbuf = consts.tile([128, NT, G], F32)
        nc.sync.dma_start(out=selbuf, in_=seldat.rearrange("nt p g -> p nt g"))
        bias2 = consts.tile([128, G], F32)
        nc.sync.dma_start(out=bias2, in_=bias2d[:, :])
        g1neg = consts.tile([128, G], F32)
        nc.sync.dma_start(out=g1neg, in_=g1negd[:, :])
        invc = consts.tile([1, G], F32)
        nc.sync.dma_start(out=invc, in_=invcd[:, :])
        ones_t = consts.tile([1, 128], F32)
        nc.vector.memset(ones_t, 1.0)
        eps_t = consts.tile([1, 1], F32)
        nc.vector.memset(eps_t, EPS)

        for b in range(B):
            j = 0
            for gl, grp in enumerate(groups):
                ntg = grp["n_tiles"]
                ps_main = mainps.tile([128, 512], F32, tag="ps_main")
                ps_statg = statps.tile([1, 2], F32, tag="ps_statg")
                for c in range(ntg):
                    xt = xpool.tile([128, 1024], F32R, tag="xt")
                    nc.sync.dma_start(out=xt, in_=xdat[b, j])
                    xf = xt.bitcast(F32)
                    st = spool.tile([128, 12], F32, tag="st")
                    nc.vector.bn_stats(out=st[:, 0:6], in_=xf[:, 0:512])
                    nc.vector.bn_stats(out=st[:, 6:12], in_=xf[:, 512:1024])
                    mv = spool.tile([128, 2], F32, tag="mv")
                    nc.vector.bn_aggr(out=mv, in_=st)
                    # mv -> [mean, E[x^2]] per bin
                    nc.vector.scalar_tensor_tensor(
                        out=mv[:, 1:2],
                        in0=mv[:, 0:1],
                        scalar=mv[:, 0:1],
                        in1=mv[:, 1:2],
                        op0=mybir.AluOpType.mult,
                        op1=mybir.AluOpType.add,
                    )
                    # group sums of (m1, m2): psum[1, 2] += sel_g.T @ mv
                    nc.tensor.matmul(
                        ps_statg[:, :],
                        selbuf[:, j, gl : gl + 1],
                        mv[:, :],
                        start=(c == 0),
                        stop=(c == ntg - 1),
                    )
                    # main: psum[128 f, 512 t] += W'.T @ x  (k = 0, 1)
                    x3 = xt.rearrange("p (k t) -> p k t", k=2)
                    for k in range(2):
                        nc.tensor.matmul(
                            ps_main[:, :],
                            wbuf[:, j, k, :],
                            x3[:, k, :],
                            start=(c == 0 and k == 0),
                            stop=(c == ntg - 1 and k == 1),
                        )
                    j += 1

                # ---- epilogue for (b, gl), all scalars on partition 0 ----
                mue = gpool.tile([1, 2], F32, tag="mue")
                tmp1 = gpool.tile([1, 1], F32, tag="tmp1")
                tmp2 = gpool.tile([1, 1], F32, tag="tmp2")
                rm = gpool.tile([1, 2], F32, tag="rm")
                # mu, e2 = psum_stats / w
                nc.scalar.activation(
                    mue[:, :],
                    ps_statg[:, :],
                    mybir.ActivationFunctionType.Copy,
                    bias=0.0,
                    scale=invc[0:1, gl : gl + 1],
                )
                # -var = mu*mu - e2
                nc.vector.scalar_tensor_tensor(
                    out=tmp1[:, :],
                    in0=mue[:, 0:1],
                    scalar=mue[:, 0:1],
                    in1=mue[:, 1:2],
                    op0=mybir.AluOpType.mult,
                    op1=mybir.AluOpType.subtract,
                )
                # sd = sqrt(var + eps)
                nc.scalar.activation(
                    tmp2[:, :],
                    tmp1[:, :],
                    mybir.ActivationFunctionType.Sqrt,
                    bias=eps_t[0:1, 0:1],
                    scale=-1.0,
                )
                # r = 1/sd ; mur = mu*r
                nc.vector.reciprocal(out=rm[:, 0:1], in_=tmp2[:, :])
                nc.vector.tensor_scalar_mul(
                    out=rm[:, 1:2], in0=mue[:, 0:1], scalar1=rm[:, 0:1]
                )
                # broadcast (r, mur) to 128 partitions via K=1 fp32 matmul
                ps_bc = bcps.tile([128, 2], F32, tag="ps_bc")
                nc.tensor.matmul(
                    ps_bc[:, :], ones_t[:, :], rm[:, :], start=True, stop=True
                )
                rb = gpool.tile([128, 2], F32, tag="rb")
                nc.vector.tensor_copy(rb, ps_bc[:, :])
                # d = g1neg * mur + bias2
                d_t = gpool.tile([128, 1], F32, tag="d_t")
                nc.vector.scalar_tensor_tensor(
                    out=d_t,
                    in0=g1neg[:, gl : gl + 1],
                    scalar=rb[:, 1:2],
                    in1=bias2[:, gl : gl + 1],
                    op0=mybir.AluOpType.mult,
                    op1=mybir.AluOpType.add,
                )
                # y = r*psum + d
                yo = yopool.tile([128, 512], F32, tag="yo")
                nc.scalar.activation(
                    yo,
                    ps_main[:, :],
                    mybir.ActivationFunctionType.Identity,
                    bias=d_t[:, 0:1],
                    scale=rb[:, 0:1],
                )
                nc.sync.dma_start(out=ydat[b, gl], in_=yo)

    _fixup_waits(nc)
    return nc


# ----------------------------------------------------------------------------
# Host staging.
# ----------------------------------------------------------------------------
def _stage_core_inputs(core, cspec, gn_gamma, gn_beta, conv_w, conv_b):
    groups = LAYOUTS[core]
    NT = sum(g["n_tiles"] for g in groups)
    G = len(groups)

    xdat = np.zeros((B, NT, 128, 1024), np.float32)
    wdat = np.zeros((NT, 2, 128, 128), np.float32)
    sel = np.zeros((NT, 128, G), np.float32)
    bias2 = np.zeros((128, G), np.float32)
    g1neg = np.zeros((128, G), np.float32)
    invc = np.zeros((1, G), np.float32)

    j = 0
    for gl, grp in enumerate(groups):
        i, s, w = grp["band"], grp["sub"], grp["w"]
        base = BANDS[i] + s * w
        gam = np.asarray(gn_gamma[i][s * w : (s + 1) * w], np.float32)  # [w]
        bet = np.asarray(gn_beta[i][s * w : (s + 1) * w], np.float32)
        Wg = np.asarray(conv_w[i][s * FC : (s + 1) * FC], np.float32)  # [FC, w, 2]
        bg = np.asarray(conv_b[i][s * FC : (s + 1) * FC], np.float32)  # [FC]
        Wp = Wg * gam[None, :, None]  # gamma-folded [FC, w, 2]
        bias2[:, gl] = bg + (Wg * bet[None, :, None]).sum(axis=(1, 2))
        g1neg[:, gl] = -Wp.sum(axis=(1, 2))
        invc[0, gl] = 1.0 / w
        for c in range(grp["n_tiles"]):
            r0, r1 = c * 128, min((c + 1) * 128, w)
            rows = r1 - r0
            for b in range(B):
                blk = cspec[b, base + r0 : base + r1]  # [rows, 512, 2]
                xdat[b, j, :rows, :] = (
                    blk.transpose(0, 2, 1).reshape(rows, 1024)
                )
            # wdat[j, k, r, f] = Wp[f, r0+r, k]
            wdat[j, :, :rows, :] = Wp[:, r0:r1, :].transpose(2, 1, 0)
            sel[j, :rows, gl] = 1.0
            j += 1
    return dict(
        xdat=xdat, wdat=wdat, seldat=sel, bias2d=bias2, g1negd=g1neg, invcd=invc
    )


def _assemble_output(core_outs):
    """core_outs: list of ydat arrays [B, G, 128, 512] -> full [B, FC, 41, T]."""
    out = np.zeros((B, FC, int(GSTART[-1]), T), np.float32)
    band_blocks = [[None] * SUB[i] for i in range(NB)]  # band -> sub -> [B,128,512]
    for core, groups in enumerate(LAYOUTS):
        for gl, grp in enumerate(groups):
            band_blocks[grp["band"]][grp["sub"]] = core_outs[core][:, gl]
    for i in range(NB):
        s_i = SUB[i]
        yb = np.stack(band_blocks[i], axis=1)  # [B, s_i, 128, 512]
        # torch order: flat (s*FC) viewed as (FC, s)
        out[:, :, GSTART[i] : GSTART[i] + s_i, :] = yb.reshape(B, s_i * FC, T).reshape(
            B, FC, s_i, T
        )
    return out


# ----------------------------------------------------------------------------
# Runner: compile the 8 per-core programs (cached), run one per device.
# ----------------------------------------------------------------------------
_CACHE = {}
_CACHE_LOCK = threading.Lock()


def _make_fn(nc):
    import jax
    import jax.core as jcore

    from concourse import bass2jax

    bass2jax.install_neuronx_cc_hook()

    in_names, out_names, out_avals, zero_outs = [], [], [], []
    partition_name = nc.partition_id_tensor.name if nc.partition_id_tensor else None
    for alloc in nc.m.functions[0].allocations:
        if not isinstance(alloc, mybir.MemoryLocationSet):
            continue
        name = alloc.memorylocations[0].name
        if alloc.kind == "ExternalInput":
            if name != partition_name:
                in_names.append(name)
        elif alloc.kind == "ExternalOutput":
            out_names.append(name)
            shape = tuple(alloc.tensor_shape)
            dtype = mybir.dt.np(alloc.dtype)
            out_avals.append(jcore.ShapedArray(shape, dtype))
            zero_outs.append(np.zeros(shape, dtype))
    n_params = len(in_names)
    all_in = list(in_names) + list(out_names)
    if partition_name is not None:
        all_in.append(partition_name)
    donate = tuple(range(n_params, n_params + len(out_names)))

    def _body(*args):
        operands = list(args)
        if partition_name is not None:
            operands.append(bass2jax.partition_id_tensor())
        outs = bass2jax._bass_exec_p.bind(
            *operands,
            out_avals=tuple(out_avals),
            in_names=tuple(all_in),
            out_names=tuple(out_names),
            lowering_input_output_aliases=(),
            sim_require_finite=False,
            sim_require_nnan=False,
            nc=nc,
        )
        return tuple(outs)

    fn = jax.jit(_body, donate_argnums=donate, keep_unused=True)
    return fn, in_names, out_names, zero_outs


def _get_programs():
    with _CACHE_LOCK:
        if "fns" not in _CACHE:
            ncs = [_build_core_program(LAYOUTS[c]) for c in range(8)]
            _CACHE["fns"] = [_make_fn(nc) for nc in ncs]
    return _CACHE["fns"]


def run_cores(core_in_maps):
    """Dispatch the 8 per-core programs on the 8 devices; returns ydat list."""
    import jax

    fns = _get_programs()
    devs = jax.devices()[:8]
    futs = []
    for i in range(8):
        fn, in_names, out_names, zero_outs = fns[i]
        args = [jax.device_put(core_in_maps[i][n], devs[i]) for n in in_names]
        args += [jax.device_put(z, devs[i]) for z in zero_outs]
        futs.append(fn(*args))
    for f in futs:
        jax.block_until_ready(f)
    return [np.asarray(futs[i][0]) for i in range(8)]


def kernel(cspec, gn_gamma, gn_beta, conv_w, conv_b):
    cspec = np.asarray(cspec, np.float32)
    in_maps = [
        _stage_core_inputs(c, cspec, gn_gamma, gn_beta, conv_w, conv_b)
        for c in range(8)
    ]
    core_outs = run_cores(in_maps)
    return _assemble_output(core_outs)


# revision 5
# speedup vs baseline: 1.2358x; 1.2344x over previous
"""BandSplitModule Trainium2 kernel.

Strategy (band/expert parallel, per spec sharding_hint): the 41 (band,
subband) groups are distributed across the 8 NeuronCores, balanced by
total HBM traffic (input bins + weights + output).  Each core runs its
own small Bass/Tile program over host-staged, densely packed inputs:

  per data tile [128 bins x 1024 (t,k)]:
    - bn_stats/bn_aggr       -> per-bin mean / E[x^2]   (VectorE, 1 pass)
    - fp32 indicator matmul  -> per-group sums of (m1, m2) in PSUM
    - 2x float32r matmuls    -> accumulate W'^T x into the group PSUM
  per (b, group) epilogue: var/rsqrt on-chip, broadcast via K=1 matmul,
    out = r * psum + (bias' - mu*r*g1)  (ScalarE), DMA to staging.

GroupNorm is folded into the conv algebraically: with W' = gamma*W,
  y = r*(W'x) + [bias + sum(beta*W) - mu*r*sum(gamma*W)]
so the normalization needs only per-group (mean, E[x^2]) scalars and the
data is read exactly once.  The host reassembles the full
[2, 128, 41, 512] output (the torch channel-order scramble is a reshape).
"""

import threading

import numpy as np

import concourse.bass as bass
import concourse.mybir as mybir
import concourse.tile as tile

SR = 44100
BANDS = [0, 1000, 4000, 8000, 16000, 20000, 22050]
SUB = [10, 12, 8, 8, 2, 1]
FC = 128
NB = len(SUB)
RANGES = [BANDS[i + 1] - BANDS[i] for i in range(NB)]
WIDTHS = [RANGES[i] // SUB[i] for i in range(NB)]
B, T = 2, 512
EPS = 1e-5
GSTART = np.concatenate([[0], np.cumsum(SUB)]).astype(int)  # global subband index base

F32 = mybir.dt.float32
F32R = mybir.dt.float32r
BF16 = mybir.dt.bfloat16


# ----------------------------------------------------------------------------
# Partition: per-core list of (band, subband) groups, balanced by traffic.
# ----------------------------------------------------------------------------
def _build_layouts():
    """Returns per-core list of dicts: {band, sub, w, n_tiles}."""
    # group inventory by band: 0:(100 x10) 1:(250 x12) 2:(500 x8) 3:(1000 x8)
    # 4:(2000 x2) 5:(2050 x1)
    per_core_bands = [
        [5, 2, 0, 0],
        [4, 2, 1, 0],
        [4, 2, 1, 0],
        [3, 3, 2, 1],
        [3, 3, 2, 1],
        [3, 3, 2, 1, 0],
        [3, 3, 2, 1, 0],
        [2, 1, 1, 1, 1, 1, 1, 0, 0, 0, 0],
    ]
    next_sub = [0] * NB
    layouts = []
    for bands in per_core_bands:
        groups = []
        for i in bands:
            s = next_sub[i]
            next_sub[i] += 1
            w = WIDTHS[i]
            groups.append(dict(band=i, sub=s, w=w, n_tiles=(w + 127) // 128))
        layouts.append(groups)
    assert next_sub == SUB, (next_sub, SUB)
    return layouts


LAYOUTS = _build_layouts()


# ----------------------------------------------------------------------------
# Workaround for this container's walrus build: it rejects instructions that
# carry multiple semaphore waits, and any wait on Drain/Matmult.  Move those
# waits onto inserted EventSemaphore instructions (one wait each).
# ----------------------------------------------------------------------------
def _fixup_waits(nc):
    def keep(ins):
        return 0 if ins.opcode in ("Drain", "Matmult") else 1

    for f in nc.m.functions:
        for bb in f.blocks:
            insts = list(bb.instructions)
            if not any(
                i.sync_info is not None
                and len(i.sync_info.on_wait) > keep(i)
                and i.opcode != "EventSemaphore"
                for i in insts
            ):
                continue
            out = []
            for ins in insts:
                si = ins.sync_info
                k = keep(ins)
                if (
                    si is not None
                    and len(si.on_wait) > k
                    and ins.opcode != "EventSemaphore"
                ):
                    waits = list(si.on_wait)
                    for j, w in enumerate(waits[k:]):
                        ev = mybir.InstEventSemaphore(name=f"{ins.name}-dw{j}")
                        ev.engine = ins.engine
                        ev.sync_info = mybir.SyncInfo(on_wait=[w], on_update=[])
                        nc.register_instruction(ev)
                        out.append(ev)
                    ins.sync_info = mybir.SyncInfo(
                        on_wait=waits[:k], on_update=list(si.on_update)
                    )
                out.append(ins)
            bb.instructions = out


# ----------------------------------------------------------------------------
# Per-core Bass program.
# ----------------------------------------------------------------------------
def _build_core_program(groups):
    import contextlib

    NT = sum(g["n_tiles"] for g in groups)
    G = len(groups)

    nc = bass.Bass("TRN2", target_bir_lowering=False)
    xdat = nc.dram_tensor("xdat", [B, NT, 128, 1024], BF16, kind="ExternalInput")
    wdat = nc.dram_tensor("wdat", [NT, 2, 128, 128], BF16, kind="ExternalInput")
    seldat = nc.dram_tensor("seldat", [NT, 128, G], F32, kind="ExternalInput")
    bias2d = nc.dram_tensor("bias2d", [128, G], F32, kind="ExternalInput")
    g1negd = nc.dram_tensor("g1negd", [128, G], F32, kind="ExternalInput")
    invcd = nc.dram_tensor("invcd", [1, G], F32, kind="ExternalInput")
    ydat = nc.dram_tensor("ydat", [B, G, 128, 512], F32, kind="ExternalOutput")

    with tile.TileContext(nc) as tc, contextlib.ExitStack() as ctx:
        consts = ctx.enter_context(tc.tile_pool(name="consts", bufs=1))
        xpool = ctx.enter_context(tc.tile_pool(name="xpool", bufs=4))
        spool = ctx.enter_context(tc.tile_pool(name="spool", bufs=4))
        yopool = ctx.enter_context(tc.tile_pool(name="yopool", bufs=3))
        gpool = ctx.enter_context(tc.tile_pool(name="gpool", bufs=2))
        mainps = ctx.enter_context(tc.tile_pool(name="mainps", bufs=3, space="PSUM"))
        statps = ctx.enter_context(tc.tile_pool(name="statps", bufs=2, space="PSUM"))
        bcps = ctx.enter_context(tc.tile_pool(name="bcps", bufs=2, space="PSUM"))

        wbuf = consts.tile([128, NT, 2, 128], BF16)
        # DRAM wdat is [NT, 2, 128(bins), 128(f)]; SBUF partition = bins.
        nc.sync.dma_start(out=wbuf, in_=wdat.rearrange("nt k p f -> p nt k f"))
        selbuf = consts.tile([128, NT, G], F32)
        nc.sync.dma_start(out=selbuf, in_=seldat.rearrange("nt p g -> p nt g"))
        bias2 = consts.tile([128, G], F32)
        nc.sync.dma_start(out=bias2, in_=bias2d[:, :])
        g1neg = consts.tile([128, G], F32)
        nc.sync.dma_start(out=g1neg, in_=g1negd[:, :])
        invc = consts.tile([1, G], F32)
        nc.sync.dma_start(out=invc, in_=invcd[:, :])
        ones_t = consts.tile([1, 128], F32)
        nc.vector.memset(ones_t, 1.0)
        eps_t = consts.tile([1, 1], F32)
        nc.vector.memset(eps_t, EPS)

        for b in range(B):
            j = 0
            for gl, grp in enumerate(groups):
                ntg = grp["n_tiles"]
                ps_main = mainps.tile([128, 512], F32, tag="ps_main")
                ps_statg = statps.tile([1, 2], F32, tag="ps_statg")
                for c in range(ntg):
                    xt = xpool.tile([128, 1024], BF16, tag="xt")
                    nc.sync.dma_start(out=xt, in_=xdat[b, j])
                    xf = xt
                    st = spool.tile([128, 12], F32, tag="st")
                    nc.vector.bn_stats(out=st[:, 0:6], in_=xf[:, 0:512])
                    nc.vector.bn_stats(out=st[:, 6:12], in_=xf[:, 512:1024])
                    mv = spool.tile([128, 2], F32, tag="mv")
                    nc.vector.bn_aggr(out=mv, in_=st)
                    # mv -> [mean, E[x^2]] per bin
                    nc.vector.scalar_tensor_tensor(
                        out=mv[:, 1:2],
                        in0=mv[:, 0:1],
                        scalar=mv[:, 0:1],
                        in1=mv[:, 1:2],
                        op0=mybir.AluOpType.mult,
                        op1=mybir.AluOpType.add,
                    )
                    # group sums of (m1, m2): psum[1, 2] += sel_g.T @ mv
                    nc.tensor.matmul(
                        ps_statg[:, :],
                        selbuf[:, j, gl : gl + 1],
                        mv[:, :],
                        start=(c == 0),
                        stop=(c == ntg - 1),
                    )
                    # main: psum[128 f, 512 t] += W'.T @ x  (k = 0, 1)
                    x3 = xt.rearrange("p (k t) -> p k t", k=2)
                    for k in range(2):
                        nc.tensor.matmul(
                            ps_main[:, :],
                            wbuf[:, j, k, :],
                            x3[:, k, :],
                            start=(c == 0 and k == 0),
                            stop=(c == ntg - 1 and k == 1),
                        )
                    j += 1

                # ---- epilogue for (b, gl), all scalars on partition 0 ----
                mue = gpool.tile([1, 2], F32, tag="mue")
                tmp1 = gpool.tile([1, 1], F32, tag="tmp1")
                tmp2 = gpool.tile([1, 1], F32, tag="tmp2")
                rm = gpool.tile([1, 2], F32, tag="rm")
                # mu, e2 = psum_stats / w
                nc.scalar.activation(
                    mue[:, :],
                    ps_statg[:, :],
                    mybir.ActivationFunctionType.Copy,
                    bias=0.0,
                    scale=invc[0:1, gl : gl + 1],
                )
                # -var = mu*mu - e2
                nc.vector.scalar_tensor_tensor(
                    out=tmp1[:, :],
                    in0=mue[:, 0:1],
                    scalar=mue[:, 0:1],
                    in1=mue[:, 1:2],
                    op0=mybir.AluOpType.mult,
                    op1=mybir.AluOpType.subtract,
                )
                # sd = sqrt(var + eps)
                nc.scalar.activation(
                    tmp2[:, :],
                    tmp1[:, :],
                    mybir.ActivationFunctionType.Sqrt,
                    bias=eps_t[0:1, 0:1],
                    scale=-1.0,
                )
                # r = 1/sd ; mur = mu*r
                nc.vector.reciprocal(out=rm[:, 0:1], in_=tmp2[:, :])
                nc.vector.tensor_scalar_mul(
                    out=rm[:, 1:2], in0=mue[:, 0:1], scalar1=rm[:, 0:1]
                )
                # broadcast (r, mur) to 128 partitions via K=1 fp32 matmul
                ps_bc = bcps.tile([128, 2], F32, tag="ps_bc")
                nc.tensor.matmul(
                    ps_bc[:, :], ones_t[:, :], rm[:, :], start=True, stop=True
                )
                rb = gpool.tile([128, 2], F32, tag="rb")
                nc.vector.tensor_copy(rb, ps_bc[:, :])
                # d = g1neg * mur + bias2
                d_t = gpool.tile([128, 1], F32, tag="d_t")
                nc.vector.scalar_tensor_tensor(
                    out=d_t,
                    in0=g1neg[:, gl : gl + 1],
                    scalar=rb[:, 1:2],
                    in1=bias2[:, gl : gl + 1],
                    op0=mybir.AluOpType.mult,
                    op1=mybir.AluOpType.add,
                )
                # y = r*psum + d
                yo = yopool.tile([128, 512], F32, tag="yo")
                nc.scalar.activation(
                    yo,
                    ps_main[:, :],
                    mybir.ActivationFunctionType.Identity,
                    bias=d_t[:, 0:1],
                    scale=rb[:, 0:1],
                )
                nc.sync.dma_start(out=ydat[b, gl], in_=yo)

    _fixup_waits(nc)
    return nc


# ----------------------------------------------------------------------------
# Host staging.
# ----------------------------------------------------------------------------
def _stage_core_inputs(core, cspec, gn_gamma, gn_beta, conv_w, conv_b):
    groups = LAYOUTS[core]
    NT = sum(g["n_tiles"] for g in groups)
    G = len(groups)

    import ml_dtypes
    bf16 = np.dtype(ml_dtypes.bfloat16)
    xdat = np.zeros((B, NT, 128, 1024), bf16)
    wdat = np.zeros((NT, 2, 128, 128), bf16)
    sel = np.zeros((NT, 128, G), np.float32)
    bias2 = np.zeros((128, G), np.float32)
    g1neg = np.zeros((128, G), np.float32)
    invc = np.zeros((1, G), np.float32)

    j = 0
    for gl, grp in enumerate(groups):
        i, s, w = grp["band"], grp["sub"], grp["w"]
        base = BANDS[i] + s * w
        gam = np.asarray(gn_gamma[i][s * w : (s + 1) * w], np.float32)  # [w]
        bet = np.asarray(gn_beta[i][s * w : (s + 1) * w], np.float32)
        Wg = np.asarray(conv_w[i][s * FC : (s + 1) * FC], np.float32)  # [FC, w, 2]
        bg = np.asarray(conv_b[i][s * FC : (s + 1) * FC], np.float32)  # [FC]
        Wp = Wg * gam[None, :, None]  # gamma-folded [FC, w, 2]
        bias2[:, gl] = bg + (Wg * bet[None, :, None]).sum(axis=(1, 2))
        g1neg[:, gl] = -Wp.sum(axis=(1, 2))
        invc[0, gl] = 1.0 / w
        for c in range(grp["n_tiles"]):
            r0, r1 = c * 128, min((c + 1) * 128, w)
            rows = r1 - r0
            for b in range(B):
                blk = cspec[b, base + r0 : base + r1]  # [rows, 512, 2]
                xdat[b, j, :rows, :] = (
                    blk.transpose(0, 2, 1).reshape(rows, 1024).astype(bf16)
                )
            # wdat[j, k, r, f] = Wp[f, r0+r, k]
            wdat[j, :, :rows, :] = Wp[:, r0:r1, :].transpose(2, 1, 0).astype(bf16)
            sel[j, :rows, gl] = 1.0
            j += 1
    return dict(
        xdat=xdat, wdat=wdat, seldat=sel, bias2d=bias2, g1negd=g1neg, invcd=invc
    )


def _assemble_output(core_outs):
    """core_outs: list of ydat arrays [B, G, 128, 512] -> full [B, FC, 41, T]."""
    out = np.zeros((B, FC, int(GSTART[-1]), T), np.float32)
    band_blocks = [[None] * SUB[i] for i in range(NB)]  # band -> sub -> [B,128,512]
    for core, groups in enumerate(LAYOUTS):
        for gl, grp in enumerate(groups):
            band_blocks[grp["band"]][grp["sub"]] = core_outs[core][:, gl]
    for i in range(NB):
        s_i = SUB[i]
        yb = np.stack(band_blocks[i], axis=1)  # [B, s_i, 128, 512]
        # torch order: flat (s*FC) viewed as (FC, s)
        out[:, :, GSTART[i] : GSTART[i] + s_i, :] = yb.reshape(B, s_i * FC, T).reshape(
            B, FC, s_i, T
        )
    return out


# ----------------------------------------------------------------------------
# Runner: compile the 8 per-core programs (cached), run one per device.
# ----------------------------------------------------------------------------
_CACHE = {}
_CACHE_LOCK = threading.Lock()


def _make_fn(nc):
    import jax
    import jax.core as jcore

    from concourse import bass2jax

    bass2jax.install_neuronx_cc_hook()

    in_names, out_names, out_avals, zero_outs = [], [], [], []
    partition_name = nc.partition_id_tensor.name if nc.partition_id_tensor else None
    for alloc in nc.m.functions[0].allocations:
        if not isinstance(alloc, mybir.MemoryLocationSet):
            continue
        name = alloc.memorylocations[0].name
        if alloc.kind == "ExternalInput":
            if name != partition_name:
                in_names.append(name)
        elif alloc.kind == "ExternalOutput":
            out_names.append(name)
            shape = tuple(alloc.tensor_shape)
            dtype = mybir.dt.np(alloc.dtype)
            out_avals.append(jcore.ShapedArray(shape, dtype))
            zero_outs.append(np.zeros(shape, dtype))
    n_params = len(in_names)
    all_in = list(in_names) + list(out_names)
    if partition_name is not None:
        all_in.append(partition_name)
    donate = tuple(range(n_params, n_params + len(out_names)))

    def _body(*args):
        operands = list(args)
        if partition_name is not None:
            operands.append(bass2jax.partition_id_tensor())
        outs = bass2jax._bass_exec_p.bind(
            *operands,
            out_avals=tuple(out_avals),
            in_names=tuple(all_in),
            out_names=tuple(out_names),
            lowering_input_output_aliases=(),
            sim_require_finite=False,
            sim_require_nnan=False,
            nc=nc,
        )
        return tuple(outs)

    fn = jax.jit(_body, donate_argnums=donate, keep_unused=True)
    return fn, in_names, out_names, zero_outs


def _get_programs():
    with _CACHE_LOCK:
        if "fns" not in _CACHE:
            ncs = [_build_core_program(LAYOUTS[c]) for c in range(8)]
            _CACHE["fns"] = [_make_fn(nc) for nc in ncs]
    return _CACHE["fns"]


def run_cores(core_in_maps):
    """Dispatch the 8 per-core programs on the 8 devices; returns ydat list."""
    import jax

    fns = _get_programs()
    devs = jax.devices()[:8]
    futs = []
    for i in range(8):
        fn, in_names, out_names, zero_outs = fns[i]
        args = [jax.device_put(core_in_maps[i][n], devs[i]) for n in in_names]
        args += [jax.device_put(z, devs[i]) for z in zero_outs]
        futs.append(fn(*args))
    for f in futs:
        jax.block_until_ready(f)
    return [np.asarray(futs[i][0]) for i in range(8)]


def kernel(cspec, gn_gamma, gn_beta, conv_w, conv_b):
    cspec = np.asarray(cspec, np.float32)
    in_maps = [
        _stage_core_inputs(c, cspec, gn_gamma, gn_beta, conv_w, conv_b)
        for c in range(8)
    ]
    core_outs = run_cores(in_maps)
    return _assemble_output(core_outs)


# revision 6
# speedup vs baseline: 1.2526x; 1.0136x over previous
"""BandSplitModule Trainium2 kernel.

Strategy (band/expert parallel, per spec sharding_hint): the 41 (band,
subband) groups are distributed across the 8 NeuronCores, balanced by
total HBM traffic (input bins + weights + output).  Each core runs its
own small Bass/Tile program over host-staged, densely packed inputs:

  per data tile [128 bins x 1024 (t,k)]:
    - bn_stats/bn_aggr       -> per-bin mean / E[x^2]   (VectorE, 1 pass)
    - fp32 indicator matmul  -> per-group sums of (m1, m2) in PSUM
    - 2x float32r matmuls    -> accumulate W'^T x into the group PSUM
  per (b, group) epilogue: var/rsqrt on-chip, broadcast via K=1 matmul,
    out = r * psum + (bias' - mu*r*g1)  (ScalarE), DMA to staging.

GroupNorm is folded into the conv algebraically: with W' = gamma*W,
  y = r*(W'x) + [bias + sum(beta*W) - mu*r*sum(gamma*W)]
so the normalization needs only per-group (mean, E[x^2]) scalars and the
data is read exactly once.  The host reassembles the full
[2, 128, 41, 512] output (the torch channel-order scramble is a reshape).
"""

import threading

import numpy as np

import concourse.bass as bass
import concourse.mybir as mybir
import concourse.tile as tile

SR = 44100
BANDS = [0, 1000, 4000, 8000, 16000, 20000, 22050]
SUB = [10, 12, 8, 8, 2, 1]
FC = 128
NB = len(SUB)
RANGES = [BANDS[i + 1] - BANDS[i] for i in range(NB)]
WIDTHS = [RANGES[i] // SUB[i] for i in range(NB)]
B, T = 2, 512
EPS = 1e-5
GSTART = np.concatenate([[0], np.cumsum(SUB)]).astype(int)  # global subband index base

F32 = mybir.dt.float32
F32R = mybir.dt.float32r
BF16 = mybir.dt.bfloat16


# ----------------------------------------------------------------------------
# Partition: per-core list of (band, subband) groups, balanced by traffic.
# ----------------------------------------------------------------------------
def _build_layouts():
    """Returns per-core list of dicts: {band, sub, w, n_tiles}."""
    # group inventory by band: 0:(100 x10) 1:(250 x12) 2:(500 x8) 3:(1000 x8)
    # 4:(2000 x2) 5:(2050 x1)
    per_core_bands = [
        [5, 2, 0, 0],
        [4, 2, 1, 0],
        [4, 2, 1, 0],
        [3, 3, 2, 1],
        [3, 3, 2, 1],
        [3, 3, 2, 1, 0],
        [3, 3, 2, 1, 0],
        [2, 1, 1, 1, 1, 1, 1, 0, 0, 0, 0],
    ]
    next_sub = [0] * NB
    layouts = []
    for bands in per_core_bands:
        groups = []
        for i in bands:
            s = next_sub[i]
            next_sub[i] += 1
            w = WIDTHS[i]
            groups.append(dict(band=i, sub=s, w=w, n_tiles=(w + 127) // 128))
        layouts.append(groups)
    assert next_sub == SUB, (next_sub, SUB)
    return layouts


LAYOUTS = _build_layouts()


# ----------------------------------------------------------------------------
# Workaround for this container's walrus build: it rejects instructions that
# carry multiple semaphore waits, and any wait on Drain/Matmult.  Move those
# waits onto inserted EventSemaphore instructions (one wait each).
# ----------------------------------------------------------------------------
def _fixup_waits(nc):
    def keep(ins):
        return 0 if ins.opcode in ("Drain", "Matmult") else 1

    for f in nc.m.functions:
        for bb in f.blocks:
            insts = list(bb.instructions)
            if not any(
                i.sync_info is not None
                and len(i.sync_info.on_wait) > keep(i)
                and i.opcode != "EventSemaphore"
                for i in insts
            ):
                continue
            out = []
            for ins in insts:
                si = ins.sync_info
                k = keep(ins)
                if (
                    si is not None
                    and len(si.on_wait) > k
                    and ins.opcode != "EventSemaphore"
                ):
                    waits = list(si.on_wait)
                    for j, w in enumerate(waits[k:]):
                        ev = mybir.InstEventSemaphore(name=f"{ins.name}-dw{j}")
                        ev.engine = ins.engine
                        ev.sync_info = mybir.SyncInfo(on_wait=[w], on_update=[])
                        nc.register_instruction(ev)
                        out.append(ev)
                    ins.sync_info = mybir.SyncInfo(
                        on_wait=waits[:k], on_update=list(si.on_update)
                    )
                out.append(ins)
            bb.instructions = out


# ----------------------------------------------------------------------------
# Per-core Bass program.
# ----------------------------------------------------------------------------
def _build_core_program(groups):
    import contextlib

    NT = sum(g["n_tiles"] for g in groups)
    G = len(groups)

    nc = bass.Bass("TRN2", target_bir_lowering=False)
    xdat = nc.dram_tensor("xdat", [B, NT, 128, 1024], BF16, kind="ExternalInput")
    wdat = nc.dram_tensor("wdat", [NT, 2, 128, 128], BF16, kind="ExternalInput")
    seldat = nc.dram_tensor("seldat", [NT, 128, G], F32, kind="ExternalInput")
    bias2d = nc.dram_tensor("bias2d", [128, G], F32, kind="ExternalInput")
    g1negd = nc.dram_tensor("g1negd", [128, G], F32, kind="ExternalInput")
    invcd = nc.dram_tensor("invcd", [1, G], F32, kind="ExternalInput")
    ydat = nc.dram_tensor("ydat", [B, G, 128, 512], F32, kind="ExternalOutput")

    with tile.TileContext(nc) as tc, contextlib.ExitStack() as ctx:
        consts = ctx.enter_context(tc.tile_pool(name="consts", bufs=1))
        xpool = ctx.enter_context(tc.tile_pool(name="xpool", bufs=6))
        spool = ctx.enter_context(tc.tile_pool(name="spool", bufs=6))
        yopool = ctx.enter_context(tc.tile_pool(name="yopool", bufs=3))
        gpool = ctx.enter_context(tc.tile_pool(name="gpool", bufs=2))
        mainps = ctx.enter_context(tc.tile_pool(name="mainps", bufs=4, space="PSUM"))
        statps = ctx.enter_context(tc.tile_pool(name="statps", bufs=2, space="PSUM"))
        bcps = ctx.enter_context(tc.tile_pool(name="bcps", bufs=2, space="PSUM"))

        wbuf = consts.tile([128, NT, 2, 128], BF16)
        # DRAM wdat is [NT, 2, 128(bins), 128(f)]; SBUF partition = bins.
        nc.sync.dma_start(out=wbuf, in_=wdat.rearrange("nt k p f -> p nt k f"))
        selbuf = consts.tile([128, NT, G], F32)
        nc.sync.dma_start(out=selbuf, in_=seldat.rearrange("nt p g -> p nt g"))
        bias2 = consts.tile([128, G], F32)
        nc.sync.dma_start(out=bias2, in_=bias2d[:, :])
        g1neg = consts.tile([128, G], F32)
        nc.sync.dma_start(out=g1neg, in_=g1negd[:, :])
        invc = consts.tile([1, G], F32)
        nc.sync.dma_start(out=invc, in_=invcd[:, :])
        ones_t = consts.tile([1, 128], F32)
        nc.vector.memset(ones_t, 1.0)
        eps_t = consts.tile([1, 1], F32)
        nc.vector.memset(eps_t, EPS)

        for b in range(B):
            j = 0
            for gl, grp in enumerate(groups):
                ntg = grp["n_tiles"]
                ps_main = mainps.tile([128, 512], F32, tag="ps_main")
                ps_statg = statps.tile([1, 2], F32, tag="ps_statg")
                for c in range(ntg):
                    xt = xpool.tile([128, 1024], BF16, tag="xt")
                    nc.sync.dma_start(out=xt, in_=xdat[b, j])
                    xf = xt
                    st = spool.tile([128, 12], F32, tag="st")
                    nc.vector.bn_stats(out=st[:, 0:6], in_=xf[:, 0:512])
                    nc.vector.bn_stats(out=st[:, 6:12], in_=xf[:, 512:1024])
                    mv = spool.tile([128, 2], F32, tag="mv")
                    nc.vector.bn_aggr(out=mv, in_=st)
                    # mv -> [mean, E[x^2]] per bin
                    nc.vector.scalar_tensor_tensor(
                        out=mv[:, 1:2],
                        in0=mv[:, 0:1],
                        scalar=mv[:, 0:1],
                        in1=mv[:, 1:2],
                        op0=mybir.AluOpType.mult,
                        op1=mybir.AluOpType.add,
                    )
                    # group sums of (m1, m2): psum[1, 2] += sel_g.T @ mv
                    nc.tensor.matmul(
                        ps_statg[:, :],
                        selbuf[:, j, gl : gl + 1],
                        mv[:, :],
                        start=(c == 0),
                        stop=(c == ntg - 1),
                    )
                    # main: psum[128 f, 512 t] += W'.T @ x  (k = 0, 1)
                    x3 = xt.rearrange("p (k t) -> p k t", k=2)
                    for k in range(2):
                        nc.tensor.matmul(
                            ps_main[:, :],
                            wbuf[:, j, k, :],
                            x3[:, k, :],
                            start=(c == 0 and k == 0),
                            stop=(c == ntg - 1 and k == 1),
                        )
                    j += 1

                # ---- epilogue for (b, gl), all scalars on partition 0 ----
                mue = gpool.tile([1, 2], F32, tag="mue")
                tmp1 = gpool.tile([1, 1], F32, tag="tmp1")
                tmp2 = gpool.tile([1, 1], F32, tag="tmp2")
                rm = gpool.tile([1, 2], F32, tag="rm")
                # mu, e2 = psum_stats / w
                nc.scalar.activation(
                    mue[:, :],
                    ps_statg[:, :],
                    mybir.ActivationFunctionType.Copy,
                    bias=0.0,
                    scale=invc[0:1, gl : gl + 1],
                )
                # -var = mu*mu - e2
                nc.vector.scalar_tensor_tensor(
                    out=tmp1[:, :],
                    in0=mue[:, 0:1],
                    scalar=mue[:, 0:1],
                    in1=mue[:, 1:2],
                    op0=mybir.AluOpType.mult,
                    op1=mybir.AluOpType.subtract,
                )
                # sd = sqrt(var + eps)
                nc.scalar.activation(
                    tmp2[:, :],
                    tmp1[:, :],
                    mybir.ActivationFunctionType.Sqrt,
                    bias=eps_t[0:1, 0:1],
                    scale=-1.0,
                )
                # r = 1/sd ; mur = mu*r
                nc.vector.reciprocal(out=rm[:, 0:1], in_=tmp2[:, :])
                nc.vector.tensor_scalar_mul(
                    out=rm[:, 1:2], in0=mue[:, 0:1], scalar1=rm[:, 0:1]
                )
                # broadcast (r, mur) to 128 partitions via K=1 fp32 matmul
                ps_bc = bcps.tile([128, 2], F32, tag="ps_bc")
                nc.tensor.matmul(
                    ps_bc[:, :], ones_t[:, :], rm[:, :], start=True, stop=True
                )
                rb = gpool.tile([128, 2], F32, tag="rb")
                nc.vector.tensor_copy(rb, ps_bc[:, :])
                # d = g1neg * mur + bias2
                d_t = gpool.tile([128, 1], F32, tag="d_t")
                nc.vector.scalar_tensor_tensor(
                    out=d_t,
                    in0=g1neg[:, gl : gl + 1],
                    scalar=rb[:, 1:2],
                    in1=bias2[:, gl : gl + 1],
                    op0=mybir.AluOpType.mult,
                    op1=mybir.AluOpType.add,
                )
                # y = r*psum + d
                yo = yopool.tile([128, 512], F32, tag="yo")
                nc.scalar.activation(
                    yo,
                    ps_main[:, :],
                    mybir.ActivationFunctionType.Identity,
                    bias=d_t[:, 0:1],
                    scale=rb[:, 0:1],
                )
                nc.sync.dma_start(out=ydat[b, gl], in_=yo)

    _fixup_waits(nc)
    return nc


# ----------------------------------------------------------------------------
# Host staging.
# ----------------------------------------------------------------------------
def _stage_core_inputs(core, cspec, gn_gamma, gn_beta, conv_w, conv_b):
    groups = LAYOUTS[core]
    NT = sum(g["n_tiles"] for g in groups)
    G = len(groups)

    import ml_dtypes
    bf16 = np.dtype(ml_dtypes.bfloat16)
    xdat = np.zeros((B, NT, 128, 1024), bf16)
    wdat = np.zeros((NT, 2, 128, 128), bf16)
    sel = np.zeros((NT, 128, G), np.float32)
    bias2 = np.zeros((128, G), np.float32)
    g1neg = np.zeros((128, G), np.float32)
    invc = np.zeros((1, G), np.float32)

    j = 0
    for gl, grp in enumerate(groups):
        i, s, w = grp["band"], grp["sub"], grp["w"]
        base = BANDS[i] + s * w
        gam = np.asarray(gn_gamma[i][s * w : (s + 1) * w], np.float32)  # [w]
        bet = np.asarray(gn_beta[i][s * w : (s + 1) * w], np.float32)
        Wg = np.asarray(conv_w[i][s * FC : (s + 1) * FC], np.float32)  # [FC, w, 2]
        bg = np.asarray(conv_b[i][s * FC : (s + 1) * FC], np.float32)  # [FC]
        Wp = Wg * gam[None, :, None]  # gamma-folded [FC, w, 2]
        bias2[:, gl] = bg + (Wg * bet[None, :, None]).sum(axis=(1, 2))
        g1neg[:, gl] = -Wp.sum(axis=(1, 2))
        invc[0, gl] = 1.0 / w
        for c in range(grp["n_tiles"]):
            r0, r1 = c * 128, min((c + 1) * 128, w)
            rows = r1 - r0
            for b in range(B):
                blk = cspec[b, base + r0 : base + r1]  # [rows, 512, 2]
                xdat[b, j, :rows, :] = (
                    blk.transpose(0, 2, 1).reshape(rows, 1024).astype(bf16)
                )
            # wdat[j, k, r, f] = Wp[f, r0+r, k]
            wdat[j, :, :rows, :] = Wp[:, r0:r1, :].transpose(2, 1, 0).astype(bf16)
            sel[j, :rows, gl] = 1.0
            j += 1
    return dict(
        xdat=xdat, wdat=wdat, seldat=sel, bias2d=bias2, g1negd=g1neg, invcd=invc
    )


def _assemble_output(core_outs):
    """core_outs: list of ydat arrays [B, G, 128, 512] -> full [B, FC, 41, T]."""
    out = np.zeros((B, FC, int(GSTART[-1]), T), np.float32)
    band_blocks = [[None] * SUB[i] for i in range(NB)]  # band -> sub -> [B,128,512]
    for core, groups in enumerate(LAYOUTS):
        for gl, grp in enumerate(groups):
            band_blocks[grp["band"]][grp["sub"]] = core_outs[core][:, gl]
    for i in range(NB):
        s_i = SUB[i]
        yb = np.stack(band_blocks[i], axis=1)  # [B, s_i, 128, 512]
        # torch order: flat (s*FC) viewed as (FC, s)
        out[:, :, GSTART[i] : GSTART[i] + s_i, :] = yb.reshape(B, s_i * FC, T).reshape(
            B, FC, s_i, T
        )
    return out


# ----------------------------------------------------------------------------
# Runner: compile the 8 per-core programs (cached), run one per device.
# ----------------------------------------------------------------------------
_CACHE = {}
_CACHE_LOCK = threading.Lock()


def _make_fn(nc):
    import jax
    import jax.core as jcore

    from concourse import bass2jax

    bass2jax.install_neuronx_cc_hook()

    in_names, out_names, out_avals, zero_outs = [], [], [], []
    partition_name = nc.partition_id_tensor.name if nc.partition_id_tensor else None
    for alloc in nc.m.functions[0].allocations:
        if not isinstance(alloc, mybir.MemoryLocationSet):
            continue
        name = alloc.memorylocations[0].name
        if alloc.kind == "ExternalInput":
            if name != partition_name:
                in_names.append(name)
        elif alloc.kind == "ExternalOutput":
            out_names.append(name)
            shape = tuple(alloc.tensor_shape)
            dtype = mybir.dt.np(alloc.dtype)
            out_avals.append(jcore.ShapedArray(shape, dtype))
            zero_outs.append(np.zeros(shape, dtype))
    n_params = len(in_names)
    all_in = list(in_names) + list(out_names)
    if partition_name is not None:
        all_in.append(partition_name)
    donate = tuple(range(n_params, n_params + len(out_names)))

    def _body(*args):
        operands = list(args)
        if partition_name is not None:
            operands.append(bass2jax.partition_id_tensor())
        outs = bass2jax._bass_exec_p.bind(
            *operands,
            out_avals=tuple(out_avals),
            in_names=tuple(all_in),
            out_names=tuple(out_names),
            lowering_input_output_aliases=(),
            sim_require_finite=False,
            sim_require_nnan=False,
            nc=nc,
        )
        return tuple(outs)

    fn = jax.jit(_body, donate_argnums=donate, keep_unused=True)
    return fn, in_names, out_names, zero_outs


def _get_programs():
    with _CACHE_LOCK:
        if "fns" not in _CACHE:
            ncs = [_build_core_program(LAYOUTS[c]) for c in range(8)]
            _CACHE["fns"] = [_make_fn(nc) for nc in ncs]
    return _CACHE["fns"]


def run_cores(core_in_maps):
    """Dispatch the 8 per-core programs on the 8 devices; returns ydat list."""
    import jax

    fns = _get_programs()
    devs = jax.devices()[:8]
    futs = []
    for i in range(8):
        fn, in_names, out_names, zero_outs = fns[i]
        args = [jax.device_put(core_in_maps[i][n], devs[i]) for n in in_names]
        args += [jax.device_put(z, devs[i]) for z in zero_outs]
        futs.append(fn(*args))
    for f in futs:
        jax.block_until_ready(f)
    return [np.asarray(futs[i][0]) for i in range(8)]


def kernel(cspec, gn_gamma, gn_beta, conv_w, conv_b):
    cspec = np.asarray(cspec, np.float32)
    in_maps = [
        _stage_core_inputs(c, cspec, gn_gamma, gn_beta, conv_w, conv_b)
        for c in range(8)
    ]
    core_outs = run_cores(in_maps)
    return _assemble_output(core_outs)


# revision 7
# speedup vs baseline: 1.3274x; 1.0597x over previous
"""BandSplitModule Trainium2 kernel.

Strategy (band/expert parallel, per spec sharding_hint): the 41 (band,
subband) groups are distributed across the 8 NeuronCores, balanced by
total HBM traffic (input bins + weights + output).  Each core runs its
own small Bass/Tile program over host-staged, densely packed inputs:

  per data tile [128 bins x 1024 (t,k)]:
    - bn_stats/bn_aggr       -> per-bin mean / E[x^2]   (VectorE, 1 pass)
    - fp32 indicator matmul  -> per-group sums of (m1, m2) in PSUM
    - 2x float32r matmuls    -> accumulate W'^T x into the group PSUM
  per (b, group) epilogue: var/rsqrt on-chip, broadcast via K=1 matmul,
    out = r * psum + (bias' - mu*r*g1)  (ScalarE), DMA to staging.

GroupNorm is folded into the conv algebraically: with W' = gamma*W,
  y = r*(W'x) + [bias + sum(beta*W) - mu*r*sum(gamma*W)]
so the normalization needs only per-group (mean, E[x^2]) scalars and the
data is read exactly once.  The host reassembles the full
[2, 128, 41, 512] output (the torch channel-order scramble is a reshape).
"""

import threading

import numpy as np

import concourse.bass as bass
import concourse.mybir as mybir
import concourse.tile as tile

SR = 44100
BANDS = [0, 1000, 4000, 8000, 16000, 20000, 22050]
SUB = [10, 12, 8, 8, 2, 1]
FC = 128
NB = len(SUB)
RANGES = [BANDS[i + 1] - BANDS[i] for i in range(NB)]
WIDTHS = [RANGES[i] // SUB[i] for i in range(NB)]
B, T = 2, 512
EPS = 1e-5
GSTART = np.concatenate([[0], np.cumsum(SUB)]).astype(int)  # global subband index base

F32 = mybir.dt.float32
F32R = mybir.dt.float32r
BF16 = mybir.dt.bfloat16


# ----------------------------------------------------------------------------
# Partition: per-core list of (band, subband) groups, balanced by traffic.
# ----------------------------------------------------------------------------
def _build_layouts():
    """Returns per-core list of dicts: {band, sub, w, n_tiles}."""
    # group inventory by band: 0:(100 x10) 1:(250 x12) 2:(500 x8) 3:(1000 x8)
    # 4:(2000 x2) 5:(2050 x1)
    per_core_bands = [
        [5, 2, 0, 0],
        [4, 2, 1, 0],
        [4, 2, 1, 0],
        [3, 3, 2, 1],
        [3, 3, 2, 1],
        [3, 3, 2, 1, 0],
        [3, 3, 2, 1, 0],
        [2, 1, 1, 1, 1, 1, 1, 0, 0, 0, 0],
    ]
    next_sub = [0] * NB
    layouts = []
    for bands in per_core_bands:
        groups = []
        for i in bands:
            s = next_sub[i]
            next_sub[i] += 1
            w = WIDTHS[i]
            groups.append(dict(band=i, sub=s, w=w, n_tiles=(w + 127) // 128))
        layouts.append(groups)
    assert next_sub == SUB, (next_sub, SUB)
    return layouts


LAYOUTS = _build_layouts()


# ----------------------------------------------------------------------------
# Workaround for this container's walrus build: it rejects instructions that
# carry multiple semaphore waits, and any wait on Drain/Matmult.  Move those
# waits onto inserted EventSemaphore instructions (one wait each).
# ----------------------------------------------------------------------------
def _fixup_waits(nc):
    def keep(ins):
        return 0 if ins.opcode in ("Drain", "Matmult") else 1

    for f in nc.m.functions:
        for bb in f.blocks:
            insts = list(bb.instructions)
            if not any(
                i.sync_info is not None
                and len(i.sync_info.on_wait) > keep(i)
                and i.opcode != "EventSemaphore"
                for i in insts
            ):
                continue
            out = []
            for ins in insts:
                si = ins.sync_info
                k = keep(ins)
                if (
                    si is not None
                    and len(si.on_wait) > k
                    and ins.opcode != "EventSemaphore"
                ):
                    waits = list(si.on_wait)
                    for j, w in enumerate(waits[k:]):
                        ev = mybir.InstEventSemaphore(name=f"{ins.name}-dw{j}")
                        ev.engine = ins.engine
                        ev.sync_info = mybir.SyncInfo(on_wait=[w], on_update=[])
                        nc.register_instruction(ev)
                        out.append(ev)
                    ins.sync_info = mybir.SyncInfo(
                        on_wait=waits[:k], on_update=list(si.on_update)
                    )
                out.append(ins)
            bb.instructions = out


# ----------------------------------------------------------------------------
# Per-core Bass program.
# ----------------------------------------------------------------------------
def _build_core_program(groups):
    import contextlib

    NT = sum(g["n_tiles"] for g in groups)
    G = len(groups)

    nc = bass.Bass("TRN2", target_bir_lowering=False)
    xdat = nc.dram_tensor("xdat", [B, NT, 128, 1024], BF16, kind="ExternalInput")
    wdat = nc.dram_tensor("wdat", [NT, 2, 128, 128], BF16, kind="ExternalInput")
    bias2d = nc.dram_tensor("bias2d", [128, G], F32, kind="ExternalInput")
    g1negd = nc.dram_tensor("g1negd", [128, G], F32, kind="ExternalInput")
    invcd = nc.dram_tensor("invcd", [1, G], F32, kind="ExternalInput")
    ydat = nc.dram_tensor("ydat", [B, G, 128, 512], F32, kind="ExternalOutput")

    with tile.TileContext(nc) as tc, contextlib.ExitStack() as ctx:
        consts = ctx.enter_context(tc.tile_pool(name="consts", bufs=1))
        xpool = ctx.enter_context(tc.tile_pool(name="xpool", bufs=6))
        spool = ctx.enter_context(tc.tile_pool(name="spool", bufs=6))
        yopool = ctx.enter_context(tc.tile_pool(name="yopool", bufs=3))
        gpool = ctx.enter_context(tc.tile_pool(name="gpool", bufs=2))
        mainps = ctx.enter_context(tc.tile_pool(name="mainps", bufs=4, space="PSUM"))
        statps = ctx.enter_context(tc.tile_pool(name="statps", bufs=2, space="PSUM"))
        bcps = ctx.enter_context(tc.tile_pool(name="bcps", bufs=2, space="PSUM"))

        wbuf = consts.tile([128, NT, 2, 128], BF16)
        # DRAM wdat is [NT, 2, 128(bins), 128(f)]; SBUF partition = bins.
        nc.sync.dma_start(out=wbuf, in_=wdat.rearrange("nt k p f -> p nt k f"))
        bias2 = consts.tile([128, G], F32)
        nc.sync.dma_start(out=bias2, in_=bias2d[:, :])
        g1neg = consts.tile([128, G], F32)
        nc.sync.dma_start(out=g1neg, in_=g1negd[:, :])
        invc = consts.tile([1, G], F32)
        nc.sync.dma_start(out=invc, in_=invcd[:, :])
        ones_t = consts.tile([1, 128], F32)
        nc.vector.memset(ones_t, 1.0)
        ones_c = consts.tile([128, 1], F32)
        nc.vector.memset(ones_c, 1.0)
        eps_t = consts.tile([1, 1], F32)
        nc.vector.memset(eps_t, EPS)

        for b in range(B):
            j = 0
            for gl, grp in enumerate(groups):
                ntg = grp["n_tiles"]
                ps_main = mainps.tile([128, 512], F32, tag="ps_main")
                ps_statg = statps.tile([1, 2], F32, tag="ps_statg")
                strip = spool.tile([128, ntg * 12], F32, tag="strip")
                for c in range(ntg):
                    xt = xpool.tile([128, 1024], BF16, tag="xt")
                    nc.sync.dma_start(out=xt, in_=xdat[b, j])
                    nc.vector.bn_stats(
                        out=strip[:, c * 12 : c * 12 + 6], in_=xt[:, 0:512]
                    )
                    nc.vector.bn_stats(
                        out=strip[:, c * 12 + 6 : c * 12 + 12], in_=xt[:, 512:1024]
                    )
                    # main: psum[128 f, 512 t] += W'.T @ x  (k = 0, 1)
                    x3 = xt.rearrange("p (k t) -> p k t", k=2)
                    for k in range(2):
                        nc.tensor.matmul(
                            ps_main[:, :],
                            wbuf[:, j, k, :],
                            x3[:, k, :],
                            start=(c == 0 and k == 0),
                            stop=(c == ntg - 1 and k == 1),
                        )
                    j += 1
                # pooled per-partition stats for the whole group (equal counts;
                # zero-padded bins only dilute, corrected by invc = ntg/w)
                mv = spool.tile([128, 2], F32, tag="mv")
                nc.vector.bn_aggr(out=mv, in_=strip)
                nc.vector.scalar_tensor_tensor(
                    out=mv[:, 1:2],
                    in0=mv[:, 0:1],
                    scalar=mv[:, 0:1],
                    in1=mv[:, 1:2],
                    op0=mybir.AluOpType.mult,
                    op1=mybir.AluOpType.add,
                )
                nc.tensor.matmul(
                    ps_statg[:, :], ones_c[:, :], mv[:, :], start=True, stop=True
                )

                # ---- epilogue for (b, gl), all scalars on partition 0 ----
                mue = gpool.tile([1, 2], F32, tag="mue")
                tmp1 = gpool.tile([1, 1], F32, tag="tmp1")
                tmp2 = gpool.tile([1, 1], F32, tag="tmp2")
                rm = gpool.tile([1, 2], F32, tag="rm")
                # mu, e2 = psum_stats / w
                nc.scalar.activation(
                    mue[:, :],
                    ps_statg[:, :],
                    mybir.ActivationFunctionType.Copy,
                    bias=0.0,
                    scale=invc[0:1, gl : gl + 1],
                )
                # -var = mu*mu - e2
                nc.vector.scalar_tensor_tensor(
                    out=tmp1[:, :],
                    in0=mue[:, 0:1],
                    scalar=mue[:, 0:1],
                    in1=mue[:, 1:2],
                    op0=mybir.AluOpType.mult,
                    op1=mybir.AluOpType.subtract,
                )
                # sd = sqrt(var + eps)
                nc.scalar.activation(
                    tmp2[:, :],
                    tmp1[:, :],
                    mybir.ActivationFunctionType.Sqrt,
                    bias=eps_t[0:1, 0:1],
                    scale=-1.0,
                )
                # r = 1/sd ; mur = mu*r
                nc.vector.reciprocal(out=rm[:, 0:1], in_=tmp2[:, :])
                nc.vector.tensor_scalar_mul(
                    out=rm[:, 1:2], in0=mue[:, 0:1], scalar1=rm[:, 0:1]
                )
                # broadcast (r, mur) to 128 partitions via K=1 fp32 matmul
                ps_bc = bcps.tile([128, 2], F32, tag="ps_bc")
                nc.tensor.matmul(
                    ps_bc[:, :], ones_t[:, :], rm[:, :], start=True, stop=True
                )
                rb = gpool.tile([128, 2], F32, tag="rb")
                nc.vector.tensor_copy(rb, ps_bc[:, :])
                # d = g1neg * mur + bias2
                d_t = gpool.tile([128, 1], F32, tag="d_t")
                nc.vector.scalar_tensor_tensor(
                    out=d_t,
                    in0=g1neg[:, gl : gl + 1],
                    scalar=rb[:, 1:2],
                    in1=bias2[:, gl : gl + 1],
                    op0=mybir.AluOpType.mult,
                    op1=mybir.AluOpType.add,
                )
                # y = r*psum + d
                yo = yopool.tile([128, 512], F32, tag="yo")
                nc.scalar.activation(
                    yo,
                    ps_main[:, :],
                    mybir.ActivationFunctionType.Identity,
                    bias=d_t[:, 0:1],
                    scale=rb[:, 0:1],
                )
                nc.sync.dma_start(out=ydat[b, gl], in_=yo)

    _fixup_waits(nc)
    return nc


# ----------------------------------------------------------------------------
# Host staging.
# ----------------------------------------------------------------------------
def _stage_core_inputs(core, cspec, gn_gamma, gn_beta, conv_w, conv_b):
    groups = LAYOUTS[core]
    NT = sum(g["n_tiles"] for g in groups)
    G = len(groups)

    import ml_dtypes
    bf16 = np.dtype(ml_dtypes.bfloat16)
    xdat = np.zeros((B, NT, 128, 1024), bf16)
    wdat = np.zeros((NT, 2, 128, 128), bf16)
    bias2 = np.zeros((128, G), np.float32)
    g1neg = np.zeros((128, G), np.float32)
    invc = np.zeros((1, G), np.float32)

    j = 0
    for gl, grp in enumerate(groups):
        i, s, w = grp["band"], grp["sub"], grp["w"]
        base = BANDS[i] + s * w
        gam = np.asarray(gn_gamma[i][s * w : (s + 1) * w], np.float32)  # [w]
        bet = np.asarray(gn_beta[i][s * w : (s + 1) * w], np.float32)
        Wg = np.asarray(conv_w[i][s * FC : (s + 1) * FC], np.float32)  # [FC, w, 2]
        bg = np.asarray(conv_b[i][s * FC : (s + 1) * FC], np.float32)  # [FC]
        Wp = Wg * gam[None, :, None]  # gamma-folded [FC, w, 2]
        bias2[:, gl] = bg + (Wg * bet[None, :, None]).sum(axis=(1, 2))
        g1neg[:, gl] = -Wp.sum(axis=(1, 2))
        invc[0, gl] = grp["n_tiles"] / w
        for c in range(grp["n_tiles"]):
            r0, r1 = c * 128, min((c + 1) * 128, w)
            rows = r1 - r0
            for b in range(B):
                blk = cspec[b, base + r0 : base + r1]  # [rows, 512, 2]
                xdat[b, j, :rows, :] = (
                    blk.transpose(0, 2, 1).reshape(rows, 1024).astype(bf16)
                )
            # wdat[j, k, r, f] = Wp[f, r0+r, k]
            wdat[j, :, :rows, :] = Wp[:, r0:r1, :].transpose(2, 1, 0).astype(bf16)
            j += 1
    return dict(xdat=xdat, wdat=wdat, bias2d=bias2, g1negd=g1neg, invcd=invc)


def _assemble_output(core_outs):
    """core_outs: list of ydat arrays [B, G, 128, 512] -> full [B, FC, 41, T]."""
    out = np.zeros((B, FC, int(GSTART[-1]), T), np.float32)
    band_blocks = [[None] * SUB[i] for i in range(NB)]  # band -> sub -> [B,128,512]
    for core, groups in enumerate(LAYOUTS):
        for gl, grp in enumerate(groups):
            band_blocks[grp["band"]][grp["sub"]] = core_outs[core][:, gl]
    for i in range(NB):
        s_i = SUB[i]
        yb = np.stack(band_blocks[i], axis=1)  # [B, s_i, 128, 512]
        # torch order: flat (s*FC) viewed as (FC, s)
        out[:, :, GSTART[i] : GSTART[i] + s_i, :] = yb.reshape(B, s_i * FC, T).reshape(
            B, FC, s_i, T
        )
    return out


# ----------------------------------------------------------------------------
# Runner: compile the 8 per-core programs (cached), run one per device.
# ----------------------------------------------------------------------------
_CACHE = {}
_CACHE_LOCK = threading.Lock()


def _make_fn(nc):
    import jax
    import jax.core as jcore

    from concourse import bass2jax

    bass2jax.install_neuronx_cc_hook()

    in_names, out_names, out_avals, zero_outs = [], [], [], []
    partition_name = nc.partition_id_tensor.name if nc.partition_id_tensor else None
    for alloc in nc.m.functions[0].allocations:
        if not isinstance(alloc, mybir.MemoryLocationSet):
            continue
        name = alloc.memorylocations[0].name
        if alloc.kind == "ExternalInput":
            if name != partition_name:
                in_names.append(name)
        elif alloc.kind == "ExternalOutput":
            out_names.append(name)
            shape = tuple(alloc.tensor_shape)
            dtype = mybir.dt.np(alloc.dtype)
            out_avals.append(jcore.ShapedArray(shape, dtype))
            zero_outs.append(np.zeros(shape, dtype))
    n_params = len(in_names)
    all_in = list(in_names) + list(out_names)
    if partition_name is not None:
        all_in.append(partition_name)
    donate = tuple(range(n_params, n_params + len(out_names)))

    def _body(*args):
        operands = list(args)
        if partition_name is not None:
            operands.append(bass2jax.partition_id_tensor())
        outs = bass2jax._bass_exec_p.bind(
            *operands,
            out_avals=tuple(out_avals),
            in_names=tuple(all_in),
            out_names=tuple(out_names),
            lowering_input_output_aliases=(),
            sim_require_finite=False,
            sim_require_nnan=False,
            nc=nc,
        )
        return tuple(outs)

    fn = jax.jit(_body, donate_argnums=donate, keep_unused=True)
    return fn, in_names, out_names, zero_outs


def _get_programs():
    with _CACHE_LOCK:
        if "fns" not in _CACHE:
            ncs = [_build_core_program(LAYOUTS[c]) for c in range(8)]
            _CACHE["fns"] = [_make_fn(nc) for nc in ncs]
    return _CACHE["fns"]


def run_cores(core_in_maps):
    """Dispatch the 8 per-core programs on the 8 devices; returns ydat list."""
    import jax

    fns = _get_programs()
    devs = jax.devices()[:8]
    futs = []
    for i in range(8):
        fn, in_names, out_names, zero_outs = fns[i]
        args = [jax.device_put(core_in_maps[i][n], devs[i]) for n in in_names]
        args += [jax.device_put(z, devs[i]) for z in zero_outs]
        futs.append(fn(*args))
    for f in futs:
        jax.block_until_ready(f)
    return [np.asarray(futs[i][0]) for i in range(8)]


def kernel(cspec, gn_gamma, gn_beta, conv_w, conv_b):
    cspec = np.asarray(cspec, np.float32)
    in_maps = [
        _stage_core_inputs(c, cspec, gn_gamma, gn_beta, conv_w, conv_b)
        for c in range(8)
    ]
    core_outs = run_cores(in_maps)
    return _assemble_output(core_outs)
